# revision 25
# baseline (speedup 1.0000x reference)
"""Trainium2 Bass kernel for nn_CFCEncoder (3-layer CfC RNN encoder).

Strategy (v2):
  - Data-parallel over batch B=512 across 8 cores (64 rows/core); weights
    replicated; the K=64-step recurrence runs locally per core.
  - Host-side: sparsity masks folded into ff1/ff2 weights; ta/tb merged into
    a single t-gate weight (exact, since ts == 1.0); per-core inputs
    pre-transposed to feature-major (768, 4096) with rows ordered (t, b).
  - All matmul operands bf16 (PSUM accumulation fp32).
  - Phase A (x-projections for layer 0) writes straight into PSUM banks;
    the recurrent L0 matmuls accumulate on top (start=False) and the
    activations read the summed gates from PSUM directly.  This removes
    the per-step DVE adds and the PSUM->SBUF evacuation copies entirely.
    A phase-A chunk covers two steps: step parity selects the partition
    half (0-63 / 64-127) of the (128, 512) chunk tiles.
  - Transposes of the new hidden states go to one shared PSUM bank (bf16
    views) and are evacuated by single packed copies on the otherwise idle
    GPSIMD engine.
  - L1/L2 PSUM packed into the two remaining banks via partition slicing.
"""

import os
import sys

for _p in ("/root/.axon_site", "/root/.axon_site/_ro/trn_rl_repo",
           "/root/.axon_site/_ro/pypackages", "/opt/trn_rl_repo"):
    if os.path.isdir(_p) and _p not in sys.path:
        sys.path.append(_p)

import numpy as np

NC = 8          # cores
B = 512         # batch
KT = 64         # timesteps
SENS = 768      # sensory features
H = [512, 256, 64]
BC = B // NC    # 64 batch rows per core
R = BC * KT     # 4096 rows per core
G0, G1, G2 = 3 * H[0], 3 * H[1], 3 * H[2]   # 1536, 768, 192 gate widths
G2P = 192       # L2 gate width (no padding needed for bf16)
NCHUNK = R // 128   # 32 phase-A chunks (2 steps each)

# junk filler matmuls emitted before dependency-gated PE groups: they run
# while the real group's semaphore is pending, keeping the PE busy so the
# HAM clock gate stays at full rate (idle windows re-throttle PE to 1.2GHz).
FILL0 = int(os.environ.get("CFC_FILL0", "8"))   # before L0 rec group
FILL1 = int(os.environ.get("CFC_FILL1", "6"))   # before L1 group
FILLT = int(os.environ.get("CFC_FILLT", "4"))   # before transposes


def split_excess_waits(nc, mybir, limit=1):
    """walrus in this toolchain rejects >1 sem wait on one instruction
    (CTRL struct). Hoist excess waits onto preceding NoOps on the same
    engine (same-engine program order preserves semantics)."""
    cnt = 0
    for fn in nc.m.functions:
        for bb in fn.blocks:
            new_insts = []
            for inst in bb.instructions:
                si = inst.sync_info
                if si is not None and si.on_wait and len(si.on_wait) > limit:
                    waits = list(si.on_wait)
                    excess, keep = waits[:-limit], waits[-limit:]
                    while excess:
                        chunk, excess = excess[:limit], excess[limit:]
                        cnt += 1
                        new_insts.append(mybir.InstNoOp(
                            name=f"I-waitsplit-{cnt}", engine=inst.engine,
                            ins=[], outs=[],
                            sync_info=mybir.SyncInfo(on_wait=chunk, on_update=[])))
                    inst.sync_info = mybir.SyncInfo(
                        on_wait=keep, on_update=list(si.on_update))
                new_insts.append(inst)
            bb.instructions = new_insts


def build_program(split_waits=True):
    import concourse.bass as bass
    import concourse.tile as tile
    import concourse.mybir as mybir

    f32 = mybir.dt.float32
    bf16 = mybir.dt.bfloat16

    Tanh = mybir.ActivationFunctionType.Tanh
    Sigm = mybir.ActivationFunctionType.Sigmoid

    nc = bass.Bass("TRN2", target_bir_lowering=False, debug=False, num_devices=NC)

    xt_d = nc.dram_tensor("xt", [SENS, R], bf16, kind="ExternalInput").ap()
    wx0_d = nc.dram_tensor("wx0", [SENS, G0], bf16, kind="ExternalInput").ap()
    wh0_d = nc.dram_tensor("wh0", [H[0], G0], bf16, kind="ExternalInput").ap()
    w1_d = nc.dram_tensor("w1", [H[0] + H[1], G1], bf16, kind="ExternalInput").ap()
    w2_d = nc.dram_tensor("w2", [H[1] + H[2], G2P], bf16, kind="ExternalInput").ap()
    id_d = nc.dram_tensor("ident", [128, 64], bf16, kind="ExternalInput").ap()
    out_d = nc.dram_tensor("out", [BC, H[2]], f32, kind="ExternalOutput").ap()

    with tile.TileContext(nc) as tc:
        with tc.tile_pool(name="pw", bufs=1) as pw, \
             tc.tile_pool(name="pact", bufs=2) as pact, \
             tc.tile_pool(name="pblend", bufs=2) as pblend, \
             tc.tile_pool(name="ph", bufs=2) as ph, \
             tc.tile_pool(name="phT", bufs=2) as phT, \
             tc.tile_pool(name="pa", bufs=2, space="PSUM") as pa, \
             tc.tile_pool(name="pg", bufs=1, space="PSUM") as pg:

            # ---- resident weights + whole xt in SBUF ----
            # issue order matters: the first phase-A chunk needs xt quarter 0
            # and wx0, so those DMAs go first.
            xtr = []
            wx0 = []
            for k in range(6):
                xtr_t = pw.tile([128, R], bf16, tag=f"xtr_{k}")
                xtr.append(xtr_t)
                wx0_t = pw.tile([128, G0], bf16, tag=f"wx0_{k}")
                wx0.append(wx0_t)
            # interleave the first xt quarter with wx0 so the k-th phase-A
            # matmul's operands arrive as early as possible
            for k in range(6):
                nc.sync.dma_start(
                    xtr[k][:, 0:256], xt_d[k * 128:(k + 1) * 128, 0:256])
                nc.sync.dma_start(wx0[k][:], wx0_d[k * 128:(k + 1) * 128, :])
            for k in range(6):
                nc.sync.dma_start(
                    xtr[k][:, 256:1024], xt_d[k * 128:(k + 1) * 128, 256:1024])
            ident = pw.tile([128, 64], bf16, tag="ident")
            nc.sync.dma_start(ident[:], id_d[:, :])
            wh0 = []
            for k in range(4):
                t = pw.tile([128, G0], bf16, tag=f"wh0_{k}")
                nc.sync.dma_start(t[:], wh0_d[k * 128:(k + 1) * 128, :])
                wh0.append(t)
            w1 = []
            for k in range(6):
                t = pw.tile([128, G1], bf16, tag=f"w1_{k}")
                nc.sync.dma_start(t[:], w1_d[k * 128:(k + 1) * 128, :])
                w1.append(t)
            w2 = []
            for k, p in enumerate((128, 128, 64)):
                t = pw.tile([p, G2P], bf16, tag=f"w2_{k}")
                nc.sync.dma_start(t[:], w2_d[k * 128:k * 128 + p, :])
                w2.append(t)
            for q in range(1, 4):
                for k in range(6):
                    nc.sync.dma_start(
                        xtr[k][:, q * 1024:(q + 1) * 1024],
                        xt_d[k * 128:(k + 1) * 128, q * 1024:(q + 1) * 1024])

            # ---- fixed PSUM banks ----
            # bank G: L1 ff-gates psum (64,512) at partitions 0-63.
            # bank Hx: byte-split: f32 cols 256-512 hold L1 t-gate psum
            #   (p0-63) and the whole L2 psum (p64-127); the first 1KB
            #   (bf16 cols 0-448 of the bf16 view) holds the hidden-state
            #   transposes (bf16).
            bankG = pg.tile([128, 512], f32, tag="bankG")
            bankH = pg.tile([128, 512], f32, tag="bankH")
            bankHb = bankH.bitcast(bf16)            # (128, 1024) bf16 view

            # filler target: free f32 cols 224-256 of bankH (bytes 896-1024,
            # untouched by transposes (0-896) and L1n1/L2 (1024-2048)).
            def fillers(n):
                for _ in range(n):
                    nc.tensor.matmul(
                        bankH[0:32, 224:256], ident[:, 0:32], ident[:, 0:32],
                        start=True, stop=True, skip_group_check=True)

            # ---- phase A: x-projections, emitted in per-gate parts so the
            # PE queue has independent work at every dependency wait ----
            def pa_alloc():
                tiles = []
                for n in range(3):
                    pa_t = pa.tile([128, 512], f32, tag=f"pa{n}")
                    tiles.append(pa_t)
                return tiles

            def pa_mms(tiles, i, n):
                with tc.high_priority(offset=-4_000_000):
                    pt = tiles[n]
                    for k in range(6):
                        nc.tensor.matmul(
                            pt[:], xtr[k][:, i * 128:(i + 1) * 128],
                            wx0[k][:, n * 512:(n + 1) * 512],
                            start=(k == 0), stop=(k == 5))

            xa = {0: pa_alloc()}
            for n in range(3):
                pa_mms(xa[0], 0, n)

            def pa_slot(t_step, slot):
                """Emit one phase-A part at a wait point. Chunk c = t//2+1:
                even step: slots 0,1 -> gates 0,1; odd step: slot 0 -> gate 2."""
                c = t_step // 2 + 1
                if c >= NCHUNK:
                    return
                if t_step % 2 == 0:
                    if slot == 0:
                        xa[c] = pa_alloc()
                        pa_mms(xa[c], c, 0)
                    else:
                        pa_mms(xa[c], c, 1)
                elif slot == 0:
                    pa_mms(xa[c], c, 2)

            h0T = h1T = h2T = None

            for t_step in range(KT):
                po = (t_step % 2) * 64
                paf = xa[t_step // 2]
                first = (t_step == 0)
                edge = t_step <= 2 or t_step >= KT - 6

                # ----- layer 0: accumulate recurrent part onto phase A -----
                if not first:
                    fillers(FILL0 if edge else 3)
                    for n in range(3):
                        for k in range(4):
                            nc.tensor.matmul(
                                paf[n][po:po + 64, :],
                                h0T[:, k * 64:(k + 1) * 64],
                                wh0[k][:, n * 512:(n + 1) * 512],
                                start=False, stop=(k == 3),
                                skip_group_check=True)

                pa_slot(t_step, 0)

                # L1 ff-gate matmuls on the previous h1 can run while the L0
                # activations/blend produce the new h0 (bankG is exclusive to
                # this group, so it may stay open across the transposes; the
                # bankH groups must not, so L1's t-gate and L2 wait)
                if not first:
                    for j in range(2):
                        nc.tensor.matmul(
                            bankG[0:64, :], h1T[:, j * 64:(j + 1) * 64],
                            w1[4 + j][:, 0:512], start=(j == 0), stop=False)

                ff1s = pact.tile([128, 512], bf16, tag="ff1s")
                nc.scalar.activation(ff1s[po:po + 64, :], paf[0][po:po + 64, :], Tanh)
                ff2s = pact.tile([128, 512], bf16, tag="ff2s")
                nc.scalar.activation(ff2s[po:po + 64, :], paf[1][po:po + 64, :], Tanh)
                sgs = pact.tile([128, 512], bf16, tag="sgs")
                nc.scalar.activation(sgs[po:po + 64, :], paf[2][po:po + 64, :], Sigm)

                d0 = pblend.tile([128, 512], bf16, tag="d0")
                nc.vector.tensor_sub(d0[po:po + 64, :], ff2s[po:po + 64, :], ff1s[po:po + 64, :])
                e0 = pblend.tile([128, 512], bf16, tag="e0")
                nc.vector.tensor_mul(e0[po:po + 64, :], d0[po:po + 64, :], sgs[po:po + 64, :])
                h0 = ph.tile([128, 512], bf16, tag="h0")
                nc.vector.tensor_add(h0[po:po + 64, :], ff1s[po:po + 64, :], e0[po:po + 64, :])

                # transpose h0 -> bankH bf16 cols 0-256; evacuate split
                # across ACT and DVE so the next L0 group starts sooner
                fillers(FILLT if edge else 2)
                for k in range(4):
                    nc.tensor.transpose(
                        bankHb[:, k * 64:(k + 1) * 64],
                        h0[po:po + 64, k * 128:(k + 1) * 128],
                        ident[po:po + 64, :])
                h0T_new = phT.tile([128, 256], bf16, tag="h0T")
                nc.scalar.copy(h0T_new[:, 0:128], bankHb[:, 0:128])
                nc.vector.tensor_copy(out=h0T_new[:, 128:256], in_=bankHb[:, 128:256])

                # ----- layer 1: new-h0 contraction chunks -----
                fillers(FILL1 if edge else 2)
                for j in range(4):
                    nc.tensor.matmul(
                        bankG[0:64, :], h0T_new[:, j * 64:(j + 1) * 64],
                        w1[j][:, 0:512], start=first and (j == 0), stop=(j == 3))
                pairs1 = []
                if not first:
                    pairs1 += [(h1T[:, j * 64:(j + 1) * 64], w1[4 + j]) for j in range(2)]
                pairs1 += [(h0T_new[:, j * 64:(j + 1) * 64], w1[j]) for j in range(4)]
                for j, (lhs, wt) in enumerate(pairs1):
                    nc.tensor.matmul(
                        bankH[0:64, 256:512], lhs, wt[:, 512:768],
                        start=(j == 0), stop=(j == len(pairs1) - 1))

                pa_slot(t_step, 1)

                ff1 = pact.tile([64, 512], bf16, tag="ff1")
                nc.scalar.activation(ff1[:], bankG[0:64, :], Tanh)
                sg1 = pact.tile([64, 256], bf16, tag="sg1")
                nc.scalar.activation(sg1[:], bankH[0:64, 256:512], Sigm)

                d1 = pblend.tile([64, 256], bf16, tag="d1")
                nc.vector.tensor_sub(d1[:], ff1[:, 256:512], ff1[:, 0:256])
                e1 = pblend.tile([64, 256], bf16, tag="e1")
                nc.vector.tensor_mul(e1[:], d1[:], sg1[:])
                h1 = ph.tile([64, 256], bf16, tag="h1")
                nc.vector.tensor_add(h1[:], ff1[:, 0:256], e1[:])

                for k in range(2):
                    nc.tensor.transpose(
                        bankHb[:, 256 + k * 64:256 + (k + 1) * 64],
                        h1[:, k * 128:(k + 1) * 128],
                        ident[0:64, :])
                h1T_new = phT.tile([128, 128], bf16, tag="h1T")
                nc.vector.tensor_copy(out=h1T_new[:], in_=bankHb[:, 256:384])

                # ----- layer 2 -----  (psum at partitions 64-127)
                pairs2 = []
                if not first:
                    pairs2 += [(h2T[:, :], w2[2])]
                pairs2 += [(h1T_new[:, j * 64:(j + 1) * 64], w2[j]) for j in range(2)]
                for j, (lhs, wt) in enumerate(pairs2):
                    nc.tensor.matmul(
                        bankH[64:128, 256:448], lhs, wt[:, :],
                        start=(j == 0), stop=(j == len(pairs2) - 1))

                ff2 = pact.tile([128, 128], bf16, tag="ff2")
                nc.scalar.activation(ff2[64:128, :], bankH[64:128, 256:384], Tanh)
                sg2 = pact.tile([128, 64], bf16, tag="sg2")
                nc.scalar.activation(sg2[64:128, :], bankH[64:128, 384:448], Sigm)

                d2 = pblend.tile([128, 64], bf16, tag="d2")
                nc.vector.tensor_sub(d2[64:128, :], ff2[64:128, 64:128], ff2[64:128, 0:64])
                e2 = pblend.tile([128, 64], bf16, tag="e2")
                nc.vector.tensor_mul(e2[64:128, :], d2[64:128, :], sg2[64:128, :])

                if t_step < KT - 1:
                    h2 = ph.tile([128, 64], bf16, tag="h2")
                    nc.vector.tensor_add(h2[64:128, :], ff2[64:128, 0:64], e2[64:128, :])
                    nc.tensor.transpose(
                        bankHb[0:64, 384:448], h2[64:128, :], ident[64:128, :])
                    h2T_new = phT.tile([64, 64], bf16, tag="h2T")
                    nc.vector.tensor_copy(out=h2T_new[:], in_=bankHb[0:64, 384:448])
                else:
                    h2f = ph.tile([128, 64], f32, tag="h2f")
                    nc.vector.tensor_add(h2f[64:128, :], ff2[64:128, 0:64], e2[64:128, :])
                    nc.sync.dma_start(out_d[:], h2f[64:128, :])
                    h2T_new = None

                h0T, h1T, h2T = h0T_new, h1T_new, h2T_new

    if split_waits:
        import concourse.mybir as mybir2
        split_excess_waits(nc, mybir2)
    return nc


def prep_inputs(base_expanded_seq, visual_seq, weights):
    """weights: dict l{li}_{name} -> np.ndarray. Returns list of per-core
    input maps."""
    import ml_dtypes
    ndt = ml_dtypes.bfloat16
    X = np.concatenate(
        [np.asarray(base_expanded_seq, np.float32),
         np.asarray(visual_seq, np.float32)], axis=-1)       # (B, K, 768)

    wmats = []
    for li in range(3):
        g = lambda n: np.asarray(weights[f"l{li}_{n}"], np.float32)
        mask = g("mask")
        f1, f2, tg = g("ff1_w") * mask, g("ff2_w") * mask, g("ta_w") + g("tb_w")
        # Gate order [ff1|ff2|t]
        wcat = np.concatenate([f1, f2, tg], axis=0)          # (3h, cat)
        wmats.append(np.ascontiguousarray(wcat.T))           # (cat, 3h)

    wx0 = np.ascontiguousarray(wmats[0][:SENS]).astype(ndt)
    wh0 = np.ascontiguousarray(wmats[0][SENS:]).astype(ndt)
    w1 = wmats[1].astype(ndt)
    w2 = wmats[2].astype(ndt)
    ident = np.concatenate([np.eye(64), np.eye(64)], axis=0).astype(ndt)

    maps = []
    for c in range(NC):
        Xc = X[c * BC:(c + 1) * BC]                          # (64, K, 768)
        rows = Xc.transpose(1, 0, 2).reshape(R, SENS)        # row = t*64 + b
        xt = np.ascontiguousarray(rows.T).astype(ndt)        # (768, 4096)
        maps.append({"xt": xt, "wx0": wx0, "wh0": wh0, "w1": w1, "w2": w2,
                     "ident": ident})
    return maps


_CACHE = {}


def run_on_device(maps, trace=False):
    from concourse.bass_utils import run_bass_kernel_spmd
    if "nc" not in _CACHE:
        _CACHE["nc"] = build_program()
    nc = _CACHE["nc"]
    kw = {}
    if trace:
        kw = dict(trace=True, trace_cores=[0])
    return run_bass_kernel_spmd(nc, maps, list(range(NC)), **kw)


def kernel(**inputs):
    base = inputs["base_expanded_seq"]
    vis = inputs["visual_seq"]
    maps = prep_inputs(base, vis, inputs)
    res = run_on_device(maps, trace=False)
    out = np.concatenate(
        [res.results[c]["out"] for c in range(NC)], axis=0)  # (512, 64)
    return out.astype(np.float32)


# revision 26
# speedup vs baseline: 1.2634x; 1.2634x over previous
"""Trainium2 Bass kernel for nn_CFCEncoder (3-layer CfC RNN encoder).

Strategy (v2):
  - Data-parallel over batch B=512 across 8 cores (64 rows/core); weights
    replicated; the K=64-step recurrence runs locally per core.
  - Host-side: sparsity masks folded into ff1/ff2 weights; ta/tb merged into
    a single t-gate weight (exact, since ts == 1.0); per-core inputs
    pre-transposed to feature-major (768, 4096) with rows ordered (t, b).
  - All matmul operands bf16 (PSUM accumulation fp32).
  - Phase A (x-projections for layer 0) writes straight into PSUM banks;
    the recurrent L0 matmuls accumulate on top (start=False) and the
    activations read the summed gates from PSUM directly.  This removes
    the per-step DVE adds and the PSUM->SBUF evacuation copies entirely.
    A phase-A chunk covers two steps: step parity selects the partition
    half (0-63 / 64-127) of the (128, 512) chunk tiles.
  - Transposes of the new hidden states go to one shared PSUM bank (bf16
    views) and are evacuated by single packed copies on the otherwise idle
    GPSIMD engine.
  - L1/L2 PSUM packed into the two remaining banks via partition slicing.
"""

import os
import sys

for _p in ("/root/.axon_site", "/root/.axon_site/_ro/trn_rl_repo",
           "/root/.axon_site/_ro/pypackages", "/opt/trn_rl_repo"):
    if os.path.isdir(_p) and _p not in sys.path:
        sys.path.append(_p)

import numpy as np

NC = 8          # cores
B = 512         # batch
KT = 64         # timesteps
SENS = 768      # sensory features
H = [512, 256, 64]
BC = B // NC    # 64 batch rows per core
R = BC * KT     # 4096 rows per core
G0, G1, G2 = 3 * H[0], 3 * H[1], 3 * H[2]   # 1536, 768, 192 gate widths
G2P = 192       # L2 gate width (no padding needed for bf16)
NCHUNK = R // 128   # 32 phase-A chunks (2 steps each)

# junk filler matmuls emitted before dependency-gated PE groups: they run
# while the real group's semaphore is pending, keeping the PE busy so the
# HAM clock gate stays at full rate (idle windows re-throttle PE to 1.2GHz).
FILL0 = int(os.environ.get("CFC_FILL0", "8"))   # before L0 rec group
FILL1 = int(os.environ.get("CFC_FILL1", "6"))   # before L1 group
FILLT = int(os.environ.get("CFC_FILLT", "4"))   # before transposes


def split_excess_waits(nc, mybir, limit=1):
    """walrus in this toolchain rejects >1 sem wait on one instruction
    (CTRL struct). Hoist excess waits onto preceding NoOps on the same
    engine (same-engine program order preserves semantics)."""
    cnt = 0
    for fn in nc.m.functions:
        for bb in fn.blocks:
            new_insts = []
            for inst in bb.instructions:
                si = inst.sync_info
                if si is not None and si.on_wait and len(si.on_wait) > limit:
                    waits = list(si.on_wait)
                    excess, keep = waits[:-limit], waits[-limit:]
                    while excess:
                        chunk, excess = excess[:limit], excess[limit:]
                        cnt += 1
                        new_insts.append(mybir.InstNoOp(
                            name=f"I-waitsplit-{cnt}", engine=inst.engine,
                            ins=[], outs=[],
                            sync_info=mybir.SyncInfo(on_wait=chunk, on_update=[])))
                    inst.sync_info = mybir.SyncInfo(
                        on_wait=keep, on_update=list(si.on_update))
                new_insts.append(inst)
            bb.instructions = new_insts


def build_program(split_waits=True):
    import concourse.bass as bass
    import concourse.tile as tile
    import concourse.mybir as mybir

    f32 = mybir.dt.float32
    bf16 = mybir.dt.bfloat16

    Tanh = mybir.ActivationFunctionType.Tanh
    Sigm = mybir.ActivationFunctionType.Sigmoid

    nc = bass.Bass("TRN2", target_bir_lowering=False, debug=False, num_devices=NC)

    xt_d = nc.dram_tensor("xt", [SENS, R], bf16, kind="ExternalInput").ap()
    wx0_d = nc.dram_tensor("wx0", [SENS, G0], bf16, kind="ExternalInput").ap()
    wh0_d = nc.dram_tensor("wh0", [H[0], G0], bf16, kind="ExternalInput").ap()
    w1_d = nc.dram_tensor("w1", [H[0] + H[1], G1], bf16, kind="ExternalInput").ap()
    w2_d = nc.dram_tensor("w2", [H[1] + H[2], G2P], bf16, kind="ExternalInput").ap()
    id_d = nc.dram_tensor("ident", [128, 64], bf16, kind="ExternalInput").ap()
    out_d = nc.dram_tensor("out", [BC, H[2]], f32, kind="ExternalOutput").ap()

    with tile.TileContext(nc) as tc:
        with tc.tile_pool(name="pw", bufs=1) as pw, \
             tc.tile_pool(name="pact", bufs=2) as pact, \
             tc.tile_pool(name="pblend", bufs=2) as pblend, \
             tc.tile_pool(name="ph", bufs=2) as ph, \
             tc.tile_pool(name="phT", bufs=2) as phT, \
             tc.tile_pool(name="pa", bufs=2, space="PSUM") as pa, \
             tc.tile_pool(name="pg", bufs=1, space="PSUM") as pg:

            # ---- resident weights + whole xt in SBUF ----
            # issue order matters: the first phase-A chunk needs xt quarter 0
            # and wx0, so those DMAs go first.
            xtr = []
            wx0 = []
            for k in range(6):
                xtr_t = pw.tile([128, R], bf16, tag=f"xtr_{k}")
                xtr.append(xtr_t)
                wx0_t = pw.tile([128, G0], bf16, tag=f"wx0_{k}")
                wx0.append(wx0_t)
            # interleave the first xt quarter with wx0 so the k-th phase-A
            # matmul's operands arrive as early as possible
            for k in range(6):
                nc.sync.dma_start(
                    xtr[k][:, 0:256], xt_d[k * 128:(k + 1) * 128, 0:256])
                nc.sync.dma_start(wx0[k][:], wx0_d[k * 128:(k + 1) * 128, :])
            for k in range(6):
                nc.sync.dma_start(
                    xtr[k][:, 256:1024], xt_d[k * 128:(k + 1) * 128, 256:1024])
            ident = pw.tile([128, 64], bf16, tag="ident")
            nc.sync.dma_start(ident[:], id_d[:, :])
            wh0 = []
            for k in range(4):
                t = pw.tile([128, G0], bf16, tag=f"wh0_{k}")
                nc.sync.dma_start(t[:], wh0_d[k * 128:(k + 1) * 128, :])
                wh0.append(t)
            w1 = []
            for k in range(6):
                t = pw.tile([128, G1], bf16, tag=f"w1_{k}")
                nc.sync.dma_start(t[:], w1_d[k * 128:(k + 1) * 128, :])
                w1.append(t)
            w2 = []
            for k, p in enumerate((128, 128, 64)):
                t = pw.tile([p, G2P], bf16, tag=f"w2_{k}")
                nc.sync.dma_start(t[:], w2_d[k * 128:k * 128 + p, :])
                w2.append(t)
            for q in range(1, 4):
                for k in range(6):
                    nc.sync.dma_start(
                        xtr[k][:, q * 1024:(q + 1) * 1024],
                        xt_d[k * 128:(k + 1) * 128, q * 1024:(q + 1) * 1024])

            # ---- fixed PSUM banks ----
            # bank G: L1 ff-gates psum (64,512) at partitions 0-63.
            # bank Hx: byte-split: f32 cols 256-512 hold L1 t-gate psum
            #   (p0-63) and the whole L2 psum (p64-127); the first 1KB
            #   (bf16 cols 0-448 of the bf16 view) holds the hidden-state
            #   transposes (bf16).
            bankG = pg.tile([128, 512], f32, tag="bankG")
            bankH = pg.tile([128, 512], f32, tag="bankH")
            bankHb = bankH.bitcast(bf16)            # (128, 1024) bf16 view

            # filler target: free f32 cols 224-256 of bankH (bytes 896-1024,
            # untouched by transposes (0-896) and L1n1/L2 (1024-2048)).
            def fillers(n):
                for _ in range(n):
                    nc.tensor.matmul(
                        bankH[0:32, 224:256], ident[:, 0:32], ident[:, 0:32],
                        start=True, stop=True, skip_group_check=True)

            # ---- phase A: x-projections, emitted in per-gate parts so the
            # PE queue has independent work at every dependency wait ----
            def pa_alloc():
                tiles = []
                for n in range(3):
                    pa_t = pa.tile([128, 512], f32, tag=f"pa{n}")
                    tiles.append(pa_t)
                return tiles

            def pa_mms(tiles, i, n):
                with tc.high_priority(offset=-4_000_000):
                    pt = tiles[n]
                    for k in range(6):
                        nc.tensor.matmul(
                            pt[:], xtr[k][:, i * 128:(i + 1) * 128],
                            wx0[k][:, n * 512:(n + 1) * 512],
                            start=(k == 0), stop=(k == 5))

            xa = {0: pa_alloc()}
            for n in range(3):
                pa_mms(xa[0], 0, n)

            def pa_slot(t_step, slot):
                """Emit one phase-A part at a wait point. Chunk c = t//2+1:
                even step: slots 0,1 -> gates 0,1; odd step: slot 0 -> gate 2."""
                c = t_step // 2 + 1
                if c >= NCHUNK:
                    return
                if t_step % 2 == 0:
                    if slot == 0:
                        xa[c] = pa_alloc()
                        pa_mms(xa[c], c, 0)
                    else:
                        pa_mms(xa[c], c, 1)
                elif slot == 0:
                    pa_mms(xa[c], c, 2)

            h0T = h1T = h2T = None

            for t_step in range(KT):
                po = (t_step % 2) * 64
                paf = xa[t_step // 2]
                first = (t_step == 0)
                edge = t_step <= 2 or t_step >= KT - 6

                # ----- layer 0: accumulate recurrent part onto phase A -----
                if not first:
                    if edge:
                        fillers(FILL0)
                    for n in range(3):
                        for k in range(4):
                            nc.tensor.matmul(
                                paf[n][po:po + 64, :],
                                h0T[:, k * 64:(k + 1) * 64],
                                wh0[k][:, n * 512:(n + 1) * 512],
                                start=False, stop=(k == 3),
                                skip_group_check=True)

                pa_slot(t_step, 0)

                # L1 ff-gate matmuls on the previous h1 can run while the L0
                # activations/blend produce the new h0 (bankG is exclusive to
                # this group, so it may stay open across the transposes; the
                # bankH groups must not, so L1's t-gate and L2 wait)
                if not first:
                    for j in range(2):
                        nc.tensor.matmul(
                            bankG[0:64, :], h1T[:, j * 64:(j + 1) * 64],
                            w1[4 + j][:, 0:512], start=(j == 0), stop=False)

                ff1s = pact.tile([128, 512], bf16, tag="ff1s")
                nc.scalar.activation(ff1s[po:po + 64, :], paf[0][po:po + 64, :], Tanh)
                ff2s = pact.tile([128, 512], bf16, tag="ff2s")
                nc.scalar.activation(ff2s[po:po + 64, :], paf[1][po:po + 64, :], Tanh)
                sgs = pact.tile([128, 512], bf16, tag="sgs")
                nc.scalar.activation(sgs[po:po + 64, :], paf[2][po:po + 64, :], Sigm)

                d0 = pblend.tile([128, 512], bf16, tag="d0")
                nc.vector.tensor_sub(d0[po:po + 64, :], ff2s[po:po + 64, :], ff1s[po:po + 64, :])
                e0 = pblend.tile([128, 512], bf16, tag="e0")
                nc.vector.tensor_mul(e0[po:po + 64, :], d0[po:po + 64, :], sgs[po:po + 64, :])
                h0 = ph.tile([128, 512], bf16, tag="h0")
                nc.vector.tensor_add(h0[po:po + 64, :], ff1s[po:po + 64, :], e0[po:po + 64, :])

                # transpose h0 -> bankH bf16 cols 0-256; evacuate split
                # across ACT and DVE so the next L0 group starts sooner
                if edge:
                    fillers(FILLT)
                for k in range(4):
                    nc.tensor.transpose(
                        bankHb[:, k * 64:(k + 1) * 64],
                        h0[po:po + 64, k * 128:(k + 1) * 128],
                        ident[po:po + 64, :])
                h0T_new = phT.tile([128, 256], bf16, tag="h0T")
                nc.scalar.copy(h0T_new[:, 0:128], bankHb[:, 0:128])
                nc.vector.tensor_copy(out=h0T_new[:, 128:256], in_=bankHb[:, 128:256])

                # ----- layer 1: new-h0 contraction chunks -----
                if edge:
                    fillers(FILL1)
                for j in range(4):
                    nc.tensor.matmul(
                        bankG[0:64, :], h0T_new[:, j * 64:(j + 1) * 64],
                        w1[j][:, 0:512], start=first and (j == 0), stop=(j == 3))
                pairs1 = []
                if not first:
                    pairs1 += [(h1T[:, j * 64:(j + 1) * 64], w1[4 + j]) for j in range(2)]
                pairs1 += [(h0T_new[:, j * 64:(j + 1) * 64], w1[j]) for j in range(4)]
                for j, (lhs, wt) in enumerate(pairs1):
                    nc.tensor.matmul(
                        bankH[0:64, 256:512], lhs, wt[:, 512:768],
                        start=(j == 0), stop=(j == len(pairs1) - 1))

                pa_slot(t_step, 1)

                ff1 = pact.tile([64, 512], bf16, tag="ff1")
                nc.scalar.activation(ff1[:], bankG[0:64, :], Tanh)
                sg1 = pact.tile([64, 256], bf16, tag="sg1")
                nc.scalar.activation(sg1[:], bankH[0:64, 256:512], Sigm)

                d1 = pblend.tile([64, 256], bf16, tag="d1")
                nc.vector.tensor_sub(d1[:], ff1[:, 256:512], ff1[:, 0:256])
                e1 = pblend.tile([64, 256], bf16, tag="e1")
                nc.vector.tensor_mul(e1[:], d1[:], sg1[:])
                h1 = ph.tile([64, 256], bf16, tag="h1")
                nc.vector.tensor_add(h1[:], ff1[:, 0:256], e1[:])

                for k in range(2):
                    nc.tensor.transpose(
                        bankHb[:, 256 + k * 64:256 + (k + 1) * 64],
                        h1[:, k * 128:(k + 1) * 128],
                        ident[0:64, :])
                h1T_new = phT.tile([128, 128], bf16, tag="h1T")
                nc.vector.tensor_copy(out=h1T_new[:], in_=bankHb[:, 256:384])

                # ----- layer 2 -----  (psum at partitions 64-127)
                pairs2 = []
                if not first:
                    pairs2 += [(h2T[:, :], w2[2])]
                pairs2 += [(h1T_new[:, j * 64:(j + 1) * 64], w2[j]) for j in range(2)]
                for j, (lhs, wt) in enumerate(pairs2):
                    nc.tensor.matmul(
                        bankH[64:128, 256:448], lhs, wt[:, :],
                        start=(j == 0), stop=(j == len(pairs2) - 1))

                ff2 = pact.tile([128, 128], bf16, tag="ff2")
                nc.scalar.activation(ff2[64:128, :], bankH[64:128, 256:384], Tanh)
                sg2 = pact.tile([128, 64], bf16, tag="sg2")
                nc.scalar.activation(sg2[64:128, :], bankH[64:128, 384:448], Sigm)

                d2 = pblend.tile([128, 64], bf16, tag="d2")
                nc.vector.tensor_sub(d2[64:128, :], ff2[64:128, 64:128], ff2[64:128, 0:64])
                e2 = pblend.tile([128, 64], bf16, tag="e2")
                nc.vector.tensor_mul(e2[64:128, :], d2[64:128, :], sg2[64:128, :])

                if t_step < KT - 1:
                    h2 = ph.tile([128, 64], bf16, tag="h2")
                    nc.vector.tensor_add(h2[64:128, :], ff2[64:128, 0:64], e2[64:128, :])
                    nc.tensor.transpose(
                        bankHb[0:64, 384:448], h2[64:128, :], ident[64:128, :])
                    h2T_new = phT.tile([64, 64], bf16, tag="h2T")
                    nc.vector.tensor_copy(out=h2T_new[:], in_=bankHb[0:64, 384:448])
                else:
                    h2f = ph.tile([128, 64], f32, tag="h2f")
                    nc.vector.tensor_add(h2f[64:128, :], ff2[64:128, 0:64], e2[64:128, :])
                    nc.sync.dma_start(out_d[:], h2f[64:128, :])
                    h2T_new = None

                h0T, h1T, h2T = h0T_new, h1T_new, h2T_new

    if split_waits:
        import concourse.mybir as mybir2
        split_excess_waits(nc, mybir2)
    return nc


def prep_inputs(base_expanded_seq, visual_seq, weights):
    """weights: dict l{li}_{name} -> np.ndarray. Returns list of per-core
    input maps."""
    import ml_dtypes
    ndt = ml_dtypes.bfloat16
    X = np.concatenate(
        [np.asarray(base_expanded_seq, np.float32),
         np.asarray(visual_seq, np.float32)], axis=-1)       # (B, K, 768)

    wmats = []
    for li in range(3):
        g = lambda n: np.asarray(weights[f"l{li}_{n}"], np.float32)
        mask = g("mask")
        f1, f2, tg = g("ff1_w") * mask, g("ff2_w") * mask, g("ta_w") + g("tb_w")
        # Gate order [ff1|ff2|t]
        wcat = np.concatenate([f1, f2, tg], axis=0)          # (3h, cat)
        wmats.append(np.ascontiguousarray(wcat.T))           # (cat, 3h)

    wx0 = np.ascontiguousarray(wmats[0][:SENS]).astype(ndt)
    wh0 = np.ascontiguousarray(wmats[0][SENS:]).astype(ndt)
    w1 = wmats[1].astype(ndt)
    w2 = wmats[2].astype(ndt)
    ident = np.concatenate([np.eye(64), np.eye(64)], axis=0).astype(ndt)

    maps = []
    for c in range(NC):
        Xc = X[c * BC:(c + 1) * BC]                          # (64, K, 768)
        rows = Xc.transpose(1, 0, 2).reshape(R, SENS)        # row = t*64 + b
        xt = np.ascontiguousarray(rows.T).astype(ndt)        # (768, 4096)
        maps.append({"xt": xt, "wx0": wx0, "wh0": wh0, "w1": w1, "w2": w2,
                     "ident": ident})
    return maps


_CACHE = {}


def run_on_device(maps, trace=False):
    from concourse.bass_utils import run_bass_kernel_spmd
    if "nc" not in _CACHE:
        _CACHE["nc"] = build_program()
    nc = _CACHE["nc"]
    kw = {}
    if trace:
        kw = dict(trace=True, trace_cores=[0])
    return run_bass_kernel_spmd(nc, maps, list(range(NC)), **kw)


def kernel(**inputs):
    base = inputs["base_expanded_seq"]
    vis = inputs["visual_seq"]
    maps = prep_inputs(base, vis, inputs)
    res = run_on_device(maps, trace=False)
    out = np.concatenate(
        [res.results[c]["out"] for c in range(NC)], axis=0)  # (512, 64)
    return out.astype(np.float32)


# revision 29
# speedup vs baseline: 1.2770x; 1.0108x over previous
"""Trainium2 Bass kernel for nn_CFCEncoder (3-layer CfC RNN encoder).

Strategy (v2):
  - Data-parallel over batch B=512 across 8 cores (64 rows/core); weights
    replicated; the K=64-step recurrence runs locally per core.
  - Host-side: sparsity masks folded into ff1/ff2 weights; ta/tb merged into
    a single t-gate weight (exact, since ts == 1.0); per-core inputs
    pre-transposed to feature-major (768, 4096) with rows ordered (t, b).
  - All matmul operands bf16 (PSUM accumulation fp32).
  - Phase A (x-projections for layer 0) writes straight into PSUM banks;
    the recurrent L0 matmuls accumulate on top (start=False) and the
    activations read the summed gates from PSUM directly.  This removes
    the per-step DVE adds and the PSUM->SBUF evacuation copies entirely.
    A phase-A chunk covers two steps: step parity selects the partition
    half (0-63 / 64-127) of the (128, 512) chunk tiles.
  - Transposes of the new hidden states go to one shared PSUM bank (bf16
    views) and are evacuated by single packed copies on the otherwise idle
    GPSIMD engine.
  - L1/L2 PSUM packed into the two remaining banks via partition slicing.
"""

import os
import sys

for _p in ("/root/.axon_site", "/root/.axon_site/_ro/trn_rl_repo",
           "/root/.axon_site/_ro/pypackages", "/opt/trn_rl_repo"):
    if os.path.isdir(_p) and _p not in sys.path:
        sys.path.append(_p)

import numpy as np

NC = 8          # cores
B = 512         # batch
KT = 64         # timesteps
SENS = 768      # sensory features
H = [512, 256, 64]
BC = B // NC    # 64 batch rows per core
R = BC * KT     # 4096 rows per core
G0, G1, G2 = 3 * H[0], 3 * H[1], 3 * H[2]   # 1536, 768, 192 gate widths
G2P = 192       # L2 gate width (no padding needed for bf16)
NCHUNK = R // 128   # 32 phase-A chunks (2 steps each)

# junk filler matmuls emitted before dependency-gated PE groups: they run
# while the real group's semaphore is pending, keeping the PE busy so the
# HAM clock gate stays at full rate (idle windows re-throttle PE to 1.2GHz).
FILL0 = int(os.environ.get("CFC_FILL0", "8"))   # before L0 rec group
FILL1 = int(os.environ.get("CFC_FILL1", "6"))   # before L1 group
FILLT = int(os.environ.get("CFC_FILLT", "4"))   # before transposes


def split_excess_waits(nc, mybir, limit=1):
    """walrus in this toolchain rejects >1 sem wait on one instruction
    (CTRL struct). Hoist excess waits onto preceding NoOps on the same
    engine (same-engine program order preserves semantics)."""
    cnt = 0
    for fn in nc.m.functions:
        for bb in fn.blocks:
            new_insts = []
            for inst in bb.instructions:
                si = inst.sync_info
                if si is not None and si.on_wait and len(si.on_wait) > limit:
                    waits = list(si.on_wait)
                    excess, keep = waits[:-limit], waits[-limit:]
                    while excess:
                        chunk, excess = excess[:limit], excess[limit:]
                        cnt += 1
                        new_insts.append(mybir.InstNoOp(
                            name=f"I-waitsplit-{cnt}", engine=inst.engine,
                            ins=[], outs=[],
                            sync_info=mybir.SyncInfo(on_wait=chunk, on_update=[])))
                    inst.sync_info = mybir.SyncInfo(
                        on_wait=keep, on_update=list(si.on_update))
                new_insts.append(inst)
            bb.instructions = new_insts


def build_program(split_waits=True):
    import concourse.bass as bass
    import concourse.tile as tile
    import concourse.mybir as mybir

    f32 = mybir.dt.float32
    bf16 = mybir.dt.bfloat16

    Tanh = mybir.ActivationFunctionType.Tanh
    Sigm = mybir.ActivationFunctionType.Sigmoid

    nc = bass.Bass("TRN2", target_bir_lowering=False, debug=False, num_devices=NC)

    xt_d = nc.dram_tensor("xt", [SENS, R], bf16, kind="ExternalInput").ap()
    wx0_d = nc.dram_tensor("wx0", [SENS, G0], bf16, kind="ExternalInput").ap()
    wh0_d = nc.dram_tensor("wh0", [H[0], G0], bf16, kind="ExternalInput").ap()
    w1_d = nc.dram_tensor("w1", [H[0] + H[1], G1], bf16, kind="ExternalInput").ap()
    w2_d = nc.dram_tensor("w2", [H[1] + H[2], G2P], bf16, kind="ExternalInput").ap()
    id_d = nc.dram_tensor("ident", [128, 64], bf16, kind="ExternalInput").ap()
    out_d = nc.dram_tensor("out", [BC, H[2]], f32, kind="ExternalOutput").ap()

    with tile.TileContext(nc) as tc:
        with tc.tile_pool(name="pw", bufs=1) as pw, \
             tc.tile_pool(name="pact", bufs=2) as pact, \
             tc.tile_pool(name="pblend", bufs=2) as pblend, \
             tc.tile_pool(name="ph", bufs=2) as ph, \
             tc.tile_pool(name="phT", bufs=2) as phT, \
             tc.tile_pool(name="pa", bufs=2, space="PSUM") as pa, \
             tc.tile_pool(name="pg", bufs=1, space="PSUM") as pg:

            # ---- resident weights + whole xt in SBUF ----
            # issue order matters: the first phase-A chunk needs xt quarter 0
            # and wx0, so those DMAs go first.
            xtr = []
            wx0 = []
            for k in range(6):
                xtr_t = pw.tile([128, R], bf16, tag=f"xtr_{k}")
                xtr.append(xtr_t)
                wx0_t = pw.tile([128, G0], bf16, tag=f"wx0_{k}")
                wx0.append(wx0_t)
            # interleave the first xt slice with the ff1-gate slice of wx0 so
            # the first phase-A accumulation group's operands arrive first
            for k in range(6):
                nc.sync.dma_start(
                    xtr[k][:, 0:256], xt_d[k * 128:(k + 1) * 128, 0:256])
                nc.sync.dma_start(
                    wx0[k][:, 0:512], wx0_d[k * 128:(k + 1) * 128, 0:512])
            for k in range(6):
                nc.sync.dma_start(
                    wx0[k][:, 512:1536], wx0_d[k * 128:(k + 1) * 128, 512:1536])
            for k in range(6):
                nc.sync.dma_start(
                    xtr[k][:, 256:1024], xt_d[k * 128:(k + 1) * 128, 256:1024])
            ident = pw.tile([128, 64], bf16, tag="ident")
            nc.sync.dma_start(ident[:], id_d[:, :])
            wh0 = []
            for k in range(4):
                t = pw.tile([128, G0], bf16, tag=f"wh0_{k}")
                nc.sync.dma_start(t[:], wh0_d[k * 128:(k + 1) * 128, :])
                wh0.append(t)
            w1 = []
            for k in range(6):
                t = pw.tile([128, G1], bf16, tag=f"w1_{k}")
                nc.sync.dma_start(t[:], w1_d[k * 128:(k + 1) * 128, :])
                w1.append(t)
            w2 = []
            for k, p in enumerate((128, 128, 64)):
                t = pw.tile([p, G2P], bf16, tag=f"w2_{k}")
                nc.sync.dma_start(t[:], w2_d[k * 128:k * 128 + p, :])
                w2.append(t)
            for q in range(1, 4):
                for k in range(6):
                    nc.sync.dma_start(
                        xtr[k][:, q * 1024:(q + 1) * 1024],
                        xt_d[k * 128:(k + 1) * 128, q * 1024:(q + 1) * 1024])

            # ---- fixed PSUM banks ----
            # bank G: L1 ff-gates psum (64,512) at partitions 0-63.
            # bank Hx: byte-split: f32 cols 256-512 hold L1 t-gate psum
            #   (p0-63) and the whole L2 psum (p64-127); the first 1KB
            #   (bf16 cols 0-448 of the bf16 view) holds the hidden-state
            #   transposes (bf16).
            bankG = pg.tile([128, 512], f32, tag="bankG")
            bankH = pg.tile([128, 512], f32, tag="bankH")
            bankHb = bankH.bitcast(bf16)            # (128, 1024) bf16 view

            # filler target: free f32 cols 224-256 of bankH (bytes 896-1024,
            # untouched by transposes (0-896) and L1n1/L2 (1024-2048)).
            def fillers(n):
                for _ in range(n):
                    nc.tensor.matmul(
                        bankH[0:32, 224:256], ident[:, 0:32], ident[:, 0:32],
                        start=True, stop=True, skip_group_check=True)

            # ---- phase A: x-projections, emitted in per-gate parts so the
            # PE queue has independent work at every dependency wait ----
            def pa_alloc():
                tiles = []
                for n in range(3):
                    pa_t = pa.tile([128, 512], f32, tag=f"pa{n}")
                    tiles.append(pa_t)
                return tiles

            def pa_mms(tiles, i, n):
                with tc.high_priority(offset=-4_000_000):
                    pt = tiles[n]
                    for k in range(6):
                        nc.tensor.matmul(
                            pt[:], xtr[k][:, i * 128:(i + 1) * 128],
                            wx0[k][:, n * 512:(n + 1) * 512],
                            start=(k == 0), stop=(k == 5))

            xa = {0: pa_alloc()}
            for n in range(3):
                pa_mms(xa[0], 0, n)

            def pa_slot(t_step, slot):
                """Emit one phase-A part at a wait point. Chunk c = t//2+1:
                even step: slots 0,1 -> gates 0,1; odd step: slot 0 -> gate 2."""
                c = t_step // 2 + 1
                if c >= NCHUNK:
                    return
                if t_step % 2 == 0:
                    if slot == 0:
                        xa[c] = pa_alloc()
                        pa_mms(xa[c], c, 0)
                    else:
                        pa_mms(xa[c], c, 1)
                elif slot == 0:
                    pa_mms(xa[c], c, 2)

            h0T = h1T = h2T = None

            for t_step in range(KT):
                po = (t_step % 2) * 64
                paf = xa[t_step // 2]
                first = (t_step == 0)
                edge = t_step <= 2 or t_step >= KT - 8

                # ----- layer 0: accumulate recurrent part onto phase A -----
                if not first:
                    if edge:
                        fillers(FILL0)
                    for n in range(3):
                        for k in range(4):
                            nc.tensor.matmul(
                                paf[n][po:po + 64, :],
                                h0T[:, k * 64:(k + 1) * 64],
                                wh0[k][:, n * 512:(n + 1) * 512],
                                start=False, stop=(k == 3),
                                skip_group_check=True)

                pa_slot(t_step, 0)

                # L1 ff-gate matmuls on the previous h1 can run while the L0
                # activations/blend produce the new h0 (bankG is exclusive to
                # this group, so it may stay open across the transposes; the
                # bankH groups must not, so L1's t-gate and L2 wait)
                if not first:
                    for j in range(2):
                        nc.tensor.matmul(
                            bankG[0:64, :], h1T[:, j * 64:(j + 1) * 64],
                            w1[4 + j][:, 0:512], start=(j == 0), stop=False)

                ff1s = pact.tile([128, 512], bf16, tag="ff1s")
                nc.scalar.activation(ff1s[po:po + 64, :], paf[0][po:po + 64, :], Tanh)
                ff2s = pact.tile([128, 512], bf16, tag="ff2s")
                nc.scalar.activation(ff2s[po:po + 64, :], paf[1][po:po + 64, :], Tanh)
                sgs = pact.tile([128, 512], bf16, tag="sgs")
                nc.scalar.activation(sgs[po:po + 64, :], paf[2][po:po + 64, :], Sigm)

                d0 = pblend.tile([128, 512], bf16, tag="d0")
                nc.vector.tensor_sub(d0[po:po + 64, :], ff2s[po:po + 64, :], ff1s[po:po + 64, :])
                e0 = pblend.tile([128, 512], bf16, tag="e0")
                nc.vector.tensor_mul(e0[po:po + 64, :], d0[po:po + 64, :], sgs[po:po + 64, :])
                h0 = ph.tile([128, 512], bf16, tag="h0")
                nc.vector.tensor_add(h0[po:po + 64, :], ff1s[po:po + 64, :], e0[po:po + 64, :])

                # transpose h0 -> bankH bf16 cols 0-256; evacuate split
                # across ACT and DVE so the next L0 group starts sooner
                if edge:
                    fillers(FILLT)
                for k in range(4):
                    nc.tensor.transpose(
                        bankHb[:, k * 64:(k + 1) * 64],
                        h0[po:po + 64, k * 128:(k + 1) * 128],
                        ident[po:po + 64, :])
                h0T_new = phT.tile([128, 256], bf16, tag="h0T")
                nc.vector.tensor_copy(out=h0T_new[:, 0:128], in_=bankHb[:, 0:128])
                nc.scalar.copy(h0T_new[:, 128:256], bankHb[:, 128:256])

                # ----- layer 1: new-h0 contraction chunks -----
                if edge:
                    fillers(FILL1)
                for j in range(4):
                    nc.tensor.matmul(
                        bankG[0:64, :], h0T_new[:, j * 64:(j + 1) * 64],
                        w1[j][:, 0:512], start=first and (j == 0), stop=(j == 3))
                pairs1 = []
                if not first:
                    pairs1 += [(h1T[:, j * 64:(j + 1) * 64], w1[4 + j]) for j in range(2)]
                pairs1 += [(h0T_new[:, j * 64:(j + 1) * 64], w1[j]) for j in range(4)]
                for j, (lhs, wt) in enumerate(pairs1):
                    nc.tensor.matmul(
                        bankH[0:64, 256:512], lhs, wt[:, 512:768],
                        start=(j == 0), stop=(j == len(pairs1) - 1))

                pa_slot(t_step, 1)

                ff1 = pact.tile([64, 512], bf16, tag="ff1")
                nc.scalar.activation(ff1[:], bankG[0:64, :], Tanh)
                sg1 = pact.tile([64, 256], bf16, tag="sg1")
                nc.scalar.activation(sg1[:], bankH[0:64, 256:512], Sigm)

                d1 = pblend.tile([64, 256], bf16, tag="d1")
                nc.vector.tensor_sub(d1[:], ff1[:, 256:512], ff1[:, 0:256])
                e1 = pblend.tile([64, 256], bf16, tag="e1")
                nc.vector.tensor_mul(e1[:], d1[:], sg1[:])
                h1 = ph.tile([64, 256], bf16, tag="h1")
                nc.vector.tensor_add(h1[:], ff1[:, 0:256], e1[:])

                for k in range(2):
                    nc.tensor.transpose(
                        bankHb[:, 256 + k * 64:256 + (k + 1) * 64],
                        h1[:, k * 128:(k + 1) * 128],
                        ident[0:64, :])
                h1T_new = phT.tile([128, 128], bf16, tag="h1T")
                nc.vector.tensor_copy(out=h1T_new[:], in_=bankHb[:, 256:384])

                # ----- layer 2 -----  (psum at partitions 64-127)
                pairs2 = []
                if not first:
                    pairs2 += [(h2T[:, :], w2[2])]
                pairs2 += [(h1T_new[:, j * 64:(j + 1) * 64], w2[j]) for j in range(2)]
                for j, (lhs, wt) in enumerate(pairs2):
                    nc.tensor.matmul(
                        bankH[64:128, 256:448], lhs, wt[:, :],
                        start=(j == 0), stop=(j == len(pairs2) - 1))

                ff2 = pact.tile([128, 128], bf16, tag="ff2")
                nc.scalar.activation(ff2[64:128, :], bankH[64:128, 256:384], Tanh)
                sg2 = pact.tile([128, 64], bf16, tag="sg2")
                nc.scalar.activation(sg2[64:128, :], bankH[64:128, 384:448], Sigm)

                d2 = pblend.tile([128, 64], bf16, tag="d2")
                nc.vector.tensor_sub(d2[64:128, :], ff2[64:128, 64:128], ff2[64:128, 0:64])
                e2 = pblend.tile([128, 64], bf16, tag="e2")
                nc.vector.tensor_mul(e2[64:128, :], d2[64:128, :], sg2[64:128, :])

                if t_step < KT - 1:
                    h2 = ph.tile([128, 64], bf16, tag="h2")
                    nc.vector.tensor_add(h2[64:128, :], ff2[64:128, 0:64], e2[64:128, :])
                    nc.tensor.transpose(
                        bankHb[0:64, 384:448], h2[64:128, :], ident[64:128, :])
                    h2T_new = phT.tile([64, 64], bf16, tag="h2T")
                    nc.vector.tensor_copy(out=h2T_new[:], in_=bankHb[0:64, 384:448])
                else:
                    h2f = ph.tile([128, 64], f32, tag="h2f")
                    nc.vector.tensor_add(h2f[64:128, :], ff2[64:128, 0:64], e2[64:128, :])
                    nc.sync.dma_start(out_d[:], h2f[64:128, :])
                    h2T_new = None

                h0T, h1T, h2T = h0T_new, h1T_new, h2T_new

    if split_waits:
        import concourse.mybir as mybir2
        split_excess_waits(nc, mybir2)
    return nc


def prep_inputs(base_expanded_seq, visual_seq, weights):
    """weights: dict l{li}_{name} -> np.ndarray. Returns list of per-core
    input maps."""
    import ml_dtypes
    ndt = ml_dtypes.bfloat16
    X = np.concatenate(
        [np.asarray(base_expanded_seq, np.float32),
         np.asarray(visual_seq, np.float32)], axis=-1)       # (B, K, 768)

    wmats = []
    for li in range(3):
        g = lambda n: np.asarray(weights[f"l{li}_{n}"], np.float32)
        mask = g("mask")
        f1, f2, tg = g("ff1_w") * mask, g("ff2_w") * mask, g("ta_w") + g("tb_w")
        # Gate order [ff1|ff2|t]
        wcat = np.concatenate([f1, f2, tg], axis=0)          # (3h, cat)
        wmats.append(np.ascontiguousarray(wcat.T))           # (cat, 3h)

    wx0 = np.ascontiguousarray(wmats[0][:SENS]).astype(ndt)
    wh0 = np.ascontiguousarray(wmats[0][SENS:]).astype(ndt)
    w1 = wmats[1].astype(ndt)
    w2 = wmats[2].astype(ndt)
    ident = np.concatenate([np.eye(64), np.eye(64)], axis=0).astype(ndt)

    maps = []
    for c in range(NC):
        Xc = X[c * BC:(c + 1) * BC]                          # (64, K, 768)
        rows = Xc.transpose(1, 0, 2).reshape(R, SENS)        # row = t*64 + b
        xt = np.ascontiguousarray(rows.T).astype(ndt)        # (768, 4096)
        maps.append({"xt": xt, "wx0": wx0, "wh0": wh0, "w1": w1, "w2": w2,
                     "ident": ident})
    return maps


_CACHE = {}


def run_on_device(maps, trace=False):
    from concourse.bass_utils import run_bass_kernel_spmd
    if "nc" not in _CACHE:
        _CACHE["nc"] = build_program()
    nc = _CACHE["nc"]
    kw = {}
    if trace:
        kw = dict(trace=True, trace_cores=[0])
    return run_bass_kernel_spmd(nc, maps, list(range(NC)), **kw)


def kernel(**inputs):
    base = inputs["base_expanded_seq"]
    vis = inputs["visual_seq"]
    maps = prep_inputs(base, vis, inputs)
    res = run_on_device(maps, trace=False)
    out = np.concatenate(
        [res.results[c]["out"] for c in range(NC)], axis=0)  # (512, 64)
    return out.astype(np.float32)


# revision 30
# speedup vs baseline: 1.2792x; 1.0017x over previous
"""Trainium2 Bass kernel for nn_CFCEncoder (3-layer CfC RNN encoder).

Strategy (v2):
  - Data-parallel over batch B=512 across 8 cores (64 rows/core); weights
    replicated; the K=64-step recurrence runs locally per core.
  - Host-side: sparsity masks folded into ff1/ff2 weights; ta/tb merged into
    a single t-gate weight (exact, since ts == 1.0); per-core inputs
    pre-transposed to feature-major (768, 4096) with rows ordered (t, b).
  - All matmul operands bf16 (PSUM accumulation fp32).
  - Phase A (x-projections for layer 0) writes straight into PSUM banks;
    the recurrent L0 matmuls accumulate on top (start=False) and the
    activations read the summed gates from PSUM directly.  This removes
    the per-step DVE adds and the PSUM->SBUF evacuation copies entirely.
    A phase-A chunk covers two steps: step parity selects the partition
    half (0-63 / 64-127) of the (128, 512) chunk tiles.
  - Transposes of the new hidden states go to one shared PSUM bank (bf16
    views) and are evacuated by single packed copies on the otherwise idle
    GPSIMD engine.
  - L1/L2 PSUM packed into the two remaining banks via partition slicing.
"""

import os
import sys

for _p in ("/root/.axon_site", "/root/.axon_site/_ro/trn_rl_repo",
           "/root/.axon_site/_ro/pypackages", "/opt/trn_rl_repo"):
    if os.path.isdir(_p) and _p not in sys.path:
        sys.path.append(_p)

import numpy as np

NC = 8          # cores
B = 512         # batch
KT = 64         # timesteps
SENS = 768      # sensory features
H = [512, 256, 64]
BC = B // NC    # 64 batch rows per core
R = BC * KT     # 4096 rows per core
G0, G1, G2 = 3 * H[0], 3 * H[1], 3 * H[2]   # 1536, 768, 192 gate widths
G2P = 192       # L2 gate width (no padding needed for bf16)
NCHUNK = R // 128   # 32 phase-A chunks (2 steps each)

# junk filler matmuls emitted before dependency-gated PE groups: they run
# while the real group's semaphore is pending, keeping the PE busy so the
# HAM clock gate stays at full rate (idle windows re-throttle PE to 1.2GHz).
FILL0 = int(os.environ.get("CFC_FILL0", "8"))   # before L0 rec group
FILL1 = int(os.environ.get("CFC_FILL1", "6"))   # before L1 group
FILLT = int(os.environ.get("CFC_FILLT", "4"))   # before transposes


def split_excess_waits(nc, mybir, limit=1):
    """walrus in this toolchain rejects >1 sem wait on one instruction
    (CTRL struct). Hoist excess waits onto preceding NoOps on the same
    engine (same-engine program order preserves semantics)."""
    cnt = 0
    for fn in nc.m.functions:
        for bb in fn.blocks:
            new_insts = []
            for inst in bb.instructions:
                si = inst.sync_info
                if si is not None and si.on_wait and len(si.on_wait) > limit:
                    waits = list(si.on_wait)
                    excess, keep = waits[:-limit], waits[-limit:]
                    while excess:
                        chunk, excess = excess[:limit], excess[limit:]
                        cnt += 1
                        new_insts.append(mybir.InstNoOp(
                            name=f"I-waitsplit-{cnt}", engine=inst.engine,
                            ins=[], outs=[],
                            sync_info=mybir.SyncInfo(on_wait=chunk, on_update=[])))
                    inst.sync_info = mybir.SyncInfo(
                        on_wait=keep, on_update=list(si.on_update))
                new_insts.append(inst)
            bb.instructions = new_insts


def build_program(split_waits=True):
    import concourse.bass as bass
    import concourse.tile as tile
    import concourse.mybir as mybir

    f32 = mybir.dt.float32
    bf16 = mybir.dt.bfloat16

    Tanh = mybir.ActivationFunctionType.Tanh
    Sigm = mybir.ActivationFunctionType.Sigmoid

    nc = bass.Bass("TRN2", target_bir_lowering=False, debug=False, num_devices=NC)

    xt_d = nc.dram_tensor("xt", [SENS, R], bf16, kind="ExternalInput").ap()
    wx0_d = nc.dram_tensor("wx0", [SENS, G0], bf16, kind="ExternalInput").ap()
    wh0_d = nc.dram_tensor("wh0", [H[0], G0], bf16, kind="ExternalInput").ap()
    w1_d = nc.dram_tensor("w1", [H[0] + H[1], G1], bf16, kind="ExternalInput").ap()
    w2_d = nc.dram_tensor("w2", [H[1] + H[2], G2P], bf16, kind="ExternalInput").ap()
    id_d = nc.dram_tensor("ident", [128, 64], bf16, kind="ExternalInput").ap()
    out_d = nc.dram_tensor("out", [BC, H[2]], f32, kind="ExternalOutput").ap()

    with tile.TileContext(nc) as tc:
        with tc.tile_pool(name="pw", bufs=1) as pw, \
             tc.tile_pool(name="pact", bufs=2) as pact, \
             tc.tile_pool(name="pblend", bufs=2) as pblend, \
             tc.tile_pool(name="ph", bufs=2) as ph, \
             tc.tile_pool(name="phT", bufs=2) as phT, \
             tc.tile_pool(name="pa", bufs=2, space="PSUM") as pa, \
             tc.tile_pool(name="pg", bufs=1, space="PSUM") as pg:

            # ---- resident weights + whole xt in SBUF ----
            # issue order matters: the first phase-A chunk needs xt quarter 0
            # and wx0, so those DMAs go first.
            xtr = []
            wx0 = []
            for k in range(6):
                xtr_t = pw.tile([128, R], bf16, tag=f"xtr_{k}")
                xtr.append(xtr_t)
                wx0_t = pw.tile([128, G0], bf16, tag=f"wx0_{k}")
                wx0.append(wx0_t)
            # interleave the first xt slice with the ff1-gate slice of wx0 so
            # the first phase-A accumulation group's operands arrive first
            for k in range(6):
                nc.sync.dma_start(
                    xtr[k][:, 0:256], xt_d[k * 128:(k + 1) * 128, 0:256])
                nc.sync.dma_start(
                    wx0[k][:, 0:512], wx0_d[k * 128:(k + 1) * 128, 0:512])
            for k in range(6):
                nc.sync.dma_start(
                    wx0[k][:, 512:1536], wx0_d[k * 128:(k + 1) * 128, 512:1536])
            for k in range(6):
                nc.sync.dma_start(
                    xtr[k][:, 256:1024], xt_d[k * 128:(k + 1) * 128, 256:1024])
            ident = pw.tile([128, 64], bf16, tag="ident")
            nc.sync.dma_start(ident[:], id_d[:, :])
            wh0 = []
            for k in range(4):
                t = pw.tile([128, G0], bf16, tag=f"wh0_{k}")
                nc.sync.dma_start(t[:], wh0_d[k * 128:(k + 1) * 128, :])
                wh0.append(t)
            w1 = []
            for k in range(6):
                t = pw.tile([128, G1], bf16, tag=f"w1_{k}")
                nc.sync.dma_start(t[:], w1_d[k * 128:(k + 1) * 128, :])
                w1.append(t)
            w2 = []
            for k, p in enumerate((128, 128, 64)):
                t = pw.tile([p, G2P], bf16, tag=f"w2_{k}")
                nc.sync.dma_start(t[:], w2_d[k * 128:k * 128 + p, :])
                w2.append(t)
            for q in range(1, 4):
                for k in range(6):
                    nc.sync.dma_start(
                        xtr[k][:, q * 1024:(q + 1) * 1024],
                        xt_d[k * 128:(k + 1) * 128, q * 1024:(q + 1) * 1024])

            # ---- fixed PSUM banks ----
            # bank G: L1 ff-gates psum (64,512) at partitions 0-63.
            # bank Hx: byte-split: f32 cols 256-512 hold L1 t-gate psum
            #   (p0-63) and the whole L2 psum (p64-127); the first 1KB
            #   (bf16 cols 0-448 of the bf16 view) holds the hidden-state
            #   transposes (bf16).
            bankG = pg.tile([128, 512], f32, tag="bankG")
            bankH = pg.tile([128, 512], f32, tag="bankH")
            bankHb = bankH.bitcast(bf16)            # (128, 1024) bf16 view

            # filler target: free f32 cols 224-256 of bankH (bytes 896-1024,
            # untouched by transposes (0-896) and L1n1/L2 (1024-2048)).
            def fillers(n):
                for _ in range(n):
                    nc.tensor.matmul(
                        bankH[0:32, 224:256], ident[:, 0:32], ident[:, 0:32],
                        start=True, stop=True, skip_group_check=True)

            # ---- phase A: x-projections, emitted in per-gate parts so the
            # PE queue has independent work at every dependency wait ----
            def pa_alloc():
                tiles = []
                for n in range(3):
                    pa_t = pa.tile([128, 512], f32, tag=f"pa{n}")
                    tiles.append(pa_t)
                return tiles

            def pa_mms(tiles, i, n):
                # default priority: the PE queue keeps these exactly at the
                # wait points they are emitted at, where they fill the
                # recurrent chain's dependency stalls
                pt = tiles[n]
                for k in range(6):
                    nc.tensor.matmul(
                        pt[:], xtr[k][:, i * 128:(i + 1) * 128],
                        wx0[k][:, n * 512:(n + 1) * 512],
                        start=(k == 0), stop=(k == 5))

            xa = {0: pa_alloc()}
            for n in range(3):
                pa_mms(xa[0], 0, n)

            def pa_slot(t_step, slot):
                """Emit one phase-A part at a wait point. Chunk c = t//2+1:
                even step: slots 0,1 -> gates 0,1; odd step: slot 0 -> gate 2."""
                c = t_step // 2 + 1
                if c >= NCHUNK:
                    return
                if t_step % 2 == 0:
                    if slot == 0:
                        xa[c] = pa_alloc()
                        pa_mms(xa[c], c, 0)
                    else:
                        pa_mms(xa[c], c, 1)
                elif slot == 0:
                    pa_mms(xa[c], c, 2)

            h0T = h1T = h2T = None

            for t_step in range(KT):
                po = (t_step % 2) * 64
                paf = xa[t_step // 2]
                first = (t_step == 0)
                edge = t_step <= 2 or t_step >= KT - 8

                # ----- layer 0: accumulate recurrent part onto phase A -----
                if not first:
                    if edge:
                        fillers(FILL0)
                    for n in range(3):
                        for k in range(4):
                            nc.tensor.matmul(
                                paf[n][po:po + 64, :],
                                h0T[:, k * 64:(k + 1) * 64],
                                wh0[k][:, n * 512:(n + 1) * 512],
                                start=False, stop=(k == 3),
                                skip_group_check=True)

                pa_slot(t_step, 0)

                # L1 ff-gate matmuls on the previous h1 can run while the L0
                # activations/blend produce the new h0 (bankG is exclusive to
                # this group, so it may stay open across the transposes; the
                # bankH groups must not, so L1's t-gate and L2 wait)
                if not first:
                    for j in range(2):
                        nc.tensor.matmul(
                            bankG[0:64, :], h1T[:, j * 64:(j + 1) * 64],
                            w1[4 + j][:, 0:512], start=(j == 0), stop=False)

                ff1s = pact.tile([128, 512], bf16, tag="ff1s")
                nc.scalar.activation(ff1s[po:po + 64, :], paf[0][po:po + 64, :], Tanh)
                ff2s = pact.tile([128, 512], bf16, tag="ff2s")
                nc.scalar.activation(ff2s[po:po + 64, :], paf[1][po:po + 64, :], Tanh)
                sgs = pact.tile([128, 512], bf16, tag="sgs")
                nc.scalar.activation(sgs[po:po + 64, :], paf[2][po:po + 64, :], Sigm)

                d0 = pblend.tile([128, 512], bf16, tag="d0")
                nc.vector.tensor_sub(d0[po:po + 64, :], ff2s[po:po + 64, :], ff1s[po:po + 64, :])
                e0 = pblend.tile([128, 512], bf16, tag="e0")
                nc.vector.tensor_mul(e0[po:po + 64, :], d0[po:po + 64, :], sgs[po:po + 64, :])
                h0 = ph.tile([128, 512], bf16, tag="h0")
                nc.vector.tensor_add(h0[po:po + 64, :], ff1s[po:po + 64, :], e0[po:po + 64, :])

                # transpose h0 -> bankH bf16 cols 0-256; evacuate split
                # across ACT and DVE so the next L0 group starts sooner
                if edge:
                    fillers(FILLT)
                for k in range(4):
                    nc.tensor.transpose(
                        bankHb[:, k * 64:(k + 1) * 64],
                        h0[po:po + 64, k * 128:(k + 1) * 128],
                        ident[po:po + 64, :])
                h0T_new = phT.tile([128, 256], bf16, tag="h0T")
                nc.vector.tensor_copy(out=h0T_new[:, 0:128], in_=bankHb[:, 0:128])
                nc.scalar.copy(h0T_new[:, 128:256], bankHb[:, 128:256])

                # ----- layer 1: new-h0 contraction chunks -----
                if edge:
                    fillers(FILL1)
                for j in range(4):
                    nc.tensor.matmul(
                        bankG[0:64, :], h0T_new[:, j * 64:(j + 1) * 64],
                        w1[j][:, 0:512], start=first and (j == 0), stop=(j == 3))
                pairs1 = []
                if not first:
                    pairs1 += [(h1T[:, j * 64:(j + 1) * 64], w1[4 + j]) for j in range(2)]
                pairs1 += [(h0T_new[:, j * 64:(j + 1) * 64], w1[j]) for j in range(4)]
                for j, (lhs, wt) in enumerate(pairs1):
                    nc.tensor.matmul(
                        bankH[0:64, 256:512], lhs, wt[:, 512:768],
                        start=(j == 0), stop=(j == len(pairs1) - 1))

                pa_slot(t_step, 1)

                ff1 = pact.tile([64, 512], bf16, tag="ff1")
                nc.scalar.activation(ff1[:], bankG[0:64, :], Tanh)
                sg1 = pact.tile([64, 256], bf16, tag="sg1")
                nc.scalar.activation(sg1[:], bankH[0:64, 256:512], Sigm)

                d1 = pblend.tile([64, 256], bf16, tag="d1")
                nc.vector.tensor_sub(d1[:], ff1[:, 256:512], ff1[:, 0:256])
                e1 = pblend.tile([64, 256], bf16, tag="e1")
                nc.vector.tensor_mul(e1[:], d1[:], sg1[:])
                h1 = ph.tile([64, 256], bf16, tag="h1")
                nc.vector.tensor_add(h1[:], ff1[:, 0:256], e1[:])

                for k in range(2):
                    nc.tensor.transpose(
                        bankHb[:, 256 + k * 64:256 + (k + 1) * 64],
                        h1[:, k * 128:(k + 1) * 128],
                        ident[0:64, :])
                h1T_new = phT.tile([128, 128], bf16, tag="h1T")
                nc.vector.tensor_copy(out=h1T_new[:], in_=bankHb[:, 256:384])

                # ----- layer 2 -----  (psum at partitions 64-127)
                pairs2 = []
                if not first:
                    pairs2 += [(h2T[:, :], w2[2])]
                pairs2 += [(h1T_new[:, j * 64:(j + 1) * 64], w2[j]) for j in range(2)]
                for j, (lhs, wt) in enumerate(pairs2):
                    nc.tensor.matmul(
                        bankH[64:128, 256:448], lhs, wt[:, :],
                        start=(j == 0), stop=(j == len(pairs2) - 1))

                ff2 = pact.tile([128, 128], bf16, tag="ff2")
                nc.scalar.activation(ff2[64:128, :], bankH[64:128, 256:384], Tanh)
                sg2 = pact.tile([128, 64], bf16, tag="sg2")
                nc.scalar.activation(sg2[64:128, :], bankH[64:128, 384:448], Sigm)

                d2 = pblend.tile([128, 64], bf16, tag="d2")
                nc.vector.tensor_sub(d2[64:128, :], ff2[64:128, 64:128], ff2[64:128, 0:64])
                e2 = pblend.tile([128, 64], bf16, tag="e2")
                nc.vector.tensor_mul(e2[64:128, :], d2[64:128, :], sg2[64:128, :])

                if t_step < KT - 1:
                    h2 = ph.tile([128, 64], bf16, tag="h2")
                    nc.vector.tensor_add(h2[64:128, :], ff2[64:128, 0:64], e2[64:128, :])
                    nc.tensor.transpose(
                        bankHb[0:64, 384:448], h2[64:128, :], ident[64:128, :])
                    h2T_new = phT.tile([64, 64], bf16, tag="h2T")
                    nc.vector.tensor_copy(out=h2T_new[:], in_=bankHb[0:64, 384:448])
                else:
                    h2f = ph.tile([128, 64], f32, tag="h2f")
                    nc.vector.tensor_add(h2f[64:128, :], ff2[64:128, 0:64], e2[64:128, :])
                    nc.sync.dma_start(out_d[:], h2f[64:128, :])
                    h2T_new = None

                h0T, h1T, h2T = h0T_new, h1T_new, h2T_new

    if split_waits:
        import concourse.mybir as mybir2
        split_excess_waits(nc, mybir2)
    return nc


def prep_inputs(base_expanded_seq, visual_seq, weights):
    """weights: dict l{li}_{name} -> np.ndarray. Returns list of per-core
    input maps."""
    import ml_dtypes
    ndt = ml_dtypes.bfloat16
    X = np.concatenate(
        [np.asarray(base_expanded_seq, np.float32),
         np.asarray(visual_seq, np.float32)], axis=-1)       # (B, K, 768)

    wmats = []
    for li in range(3):
        g = lambda n: np.asarray(weights[f"l{li}_{n}"], np.float32)
        mask = g("mask")
        f1, f2, tg = g("ff1_w") * mask, g("ff2_w") * mask, g("ta_w") + g("tb_w")
        # Gate order [ff1|ff2|t]
        wcat = np.concatenate([f1, f2, tg], axis=0)          # (3h, cat)
        wmats.append(np.ascontiguousarray(wcat.T))           # (cat, 3h)

    wx0 = np.ascontiguousarray(wmats[0][:SENS]).astype(ndt)
    wh0 = np.ascontiguousarray(wmats[0][SENS:]).astype(ndt)
    w1 = wmats[1].astype(ndt)
    w2 = wmats[2].astype(ndt)
    ident = np.concatenate([np.eye(64), np.eye(64)], axis=0).astype(ndt)

    maps = []
    for c in range(NC):
        Xc = X[c * BC:(c + 1) * BC]                          # (64, K, 768)
        rows = Xc.transpose(1, 0, 2).reshape(R, SENS)        # row = t*64 + b
        xt = np.ascontiguousarray(rows.T).astype(ndt)        # (768, 4096)
        maps.append({"xt": xt, "wx0": wx0, "wh0": wh0, "w1": w1, "w2": w2,
                     "ident": ident})
    return maps


_CACHE = {}


def run_on_device(maps, trace=False):
    from concourse.bass_utils import run_bass_kernel_spmd
    if "nc" not in _CACHE:
        _CACHE["nc"] = build_program()
    nc = _CACHE["nc"]
    kw = {}
    if trace:
        kw = dict(trace=True, trace_cores=[0])
    return run_bass_kernel_spmd(nc, maps, list(range(NC)), **kw)


def kernel(**inputs):
    base = inputs["base_expanded_seq"]
    vis = inputs["visual_seq"]
    maps = prep_inputs(base, vis, inputs)
    res = run_on_device(maps, trace=False)
    out = np.concatenate(
        [res.results[c]["out"] for c in range(NC)], axis=0)  # (512, 64)
    return out.astype(np.float32)
